# revision 1
# baseline (speedup 1.0000x reference)
"""Trainium2 Bass kernel for nn_LocalNetwork (avgpool3d -> 3x LocallyConnected1D -> upsample3d).

Sharding: pure data parallelism — batch 256 split as 32 per core across 8 cores.

Per-core layout strategy (B_loc=32 batches, processed in 4 groups of 8):
  partition p = (b_loc, ho)   [8 x 16 = 128 partitions]
  The 3x4x4 avg-pool reduces (di, hs, ws) entirely along the free axis.
  All three locally-connected conv passes become free-axis ops with
  per-(partition, free-elem) weight tiles (precomputed on host):
    - depth pass:  conv along 'do' (free axis, zero-padded tile)
    - lon pass:    conv along 'wo' (free axis, zero-padded tile)
    - lat pass:    conv along 'ho' (partition axis) -> +/-1 partition shifts
                   done on the TensorEngine with constant 0/1 shift matrices
  Upsample 3x4x4 is materialized on-chip (ScalarE copies) so the store DMA
  writes 2KB-contiguous DRAM runs.
"""

import numpy as np

import concourse.bass as bass
import concourse.mybir as mybir
from concourse import bacc
from concourse.bass_utils import run_bass_kernel_spmd
from concourse.tile import TileContext

F32 = mybir.dt.float32
ADD = mybir.AluOpType.add
MULT = mybir.AluOpType.mult
RELU = mybir.ActivationFunctionType.Relu

N_CORES = 8
B = 256
B_CORE = 32          # batches per core
G = 4                # groups per core
B_GRP = 8            # batches per group
CORE_ELEMS = B_CORE * 15 * 64 * 128  # 3,932,160
BSTRIDE = 15 * 64 * 128              # 122,880

# const tile columns
CW = 12 * 160 + 2 * 128  # 2176


def _pack_consts(w_depth, b_depth, w_lon, b_lon, w_lat, b_lat) -> np.ndarray:
    """[128, 2176] f32: 12 x [128,160] conv weight tiles + 2 x [128,128] shift mats.

    Weight tiles: partition p=(bl*16+ho), free=(do*32+wo); replicated over bl.
    Depth weights pre-scaled by 1/48 to fold in the avg-pool mean.
    """
    ho = np.arange(16)[:, None, None]
    do = np.arange(5)[None, :, None]
    wo = np.arange(32)[None, None, :]
    ld = wo * 112 + ho * 7 + (do + 1)     # depth seq index (16,5,32)
    ll = do * 544 + ho * 34 + (wo + 1)    # lon
    lt = do * 576 + wo * 18 + (ho + 1)    # lat

    def tile(vec, idx):
        t = np.broadcast_to(vec[idx][None], (8, 16, 5, 32))
        return t.reshape(128, 160)

    cols = []
    for j in range(3):
        cols.append(tile(np.asarray(w_depth)[:, j] / 48.0, ld))
    cols.append(tile(np.asarray(b_depth), ld))
    for j in range(3):
        cols.append(tile(np.asarray(w_lon)[:, j], ll))
    cols.append(tile(np.asarray(b_lon), ll))
    for j in range(3):
        cols.append(tile(np.asarray(w_lat)[:, j], lt))
    cols.append(tile(np.asarray(b_lat), lt))

    up = np.zeros((128, 128), np.float32)   # S_up[p]=Y[p+1] (next ho), 0 at ho=15
    dn = np.zeros((128, 128), np.float32)   # S_dn[p]=Y[p-1], 0 at ho=0
    for p in range(128):
        if p % 16 != 15:
            up[p + 1, p] = 1.0
        if p % 16 != 0:
            dn[p - 1, p] = 1.0
    cols.append(up)
    cols.append(dn)
    return np.ascontiguousarray(np.concatenate(cols, axis=1), dtype=np.float32)


# DRAM access pattern for the load/store of one batch (16 partitions = ho/j):
# [[ho/j: 512, 16], [(do,di)/(i,a): 8192, 15], [(hs,ww)/(c,we): 1, 512]]
# DMA APs are limited to 3 dims, so each batch-group issues 8 of these
# (partition-disjoint, so Tile runs them concurrently).
_B_AP = [[512, 16], [8192, 15], [1, 512]]


def build_nc(reps: int = 1) -> bass.Bass:
    """reps>1 repeats the whole per-core computation (idempotent) so HW time
    can be measured as a wall-clock difference between two rep counts."""
    nc = bacc.Bacc("TRN2", target_bir_lowering=False, debug=False)
    x = nc.dram_tensor("x", [CORE_ELEMS], F32, kind="ExternalInput")
    cst = nc.dram_tensor("c", [128, CW], F32, kind="ExternalInput")
    y = nc.dram_tensor("y", [CORE_ELEMS], F32, kind="ExternalOutput")

    with TileContext(nc) as tc:
        with (
            tc.tile_pool(name="cpool", bufs=1) as cpool,
            tc.tile_pool(name="inp", bufs=2) as inp,
            tc.tile_pool(name="outp", bufs=2) as outp,
            tc.tile_pool(name="work", bufs=2) as work,
            tc.tile_pool(name="psum", bufs=2, space="PSUM") as psum,
        ):
            C = cpool.tile([128, CW], F32)
            nc.sync.dma_start(C[:], cst[:])
            wslice = lambda i: C[:, i * 160:(i + 1) * 160]
            wd0, wd1, wd2, bd = (wslice(i) for i in range(4))
            vl0, vl1, vl2, blon = (wslice(i) for i in range(4, 8))
            ul0, ul1, ul2, blat = (wslice(i) for i in range(8, 12))
            shift_up = C[:, 1920:2048]
            shift_dn = C[:, 2048:2176]

            for g in range(G * reps):
                g = g % G
                off = g * B_GRP * BSTRIDE

                # ---- load: X[p=(bl,ho), free=(do,di,(hs,ww))] ----
                X = inp.tile([128, 7680], F32)
                for bl in range(B_GRP):
                    nc.sync.dma_start(
                        X[bl * 16:(bl + 1) * 16, :],
                        bass.AP(x, off + bl * BSTRIDE, list(_B_AP)))

                # ---- W-pool: sum groups of 4 along ww -> P1 (do,di,hs,wo) ----
                Xr = X[:].rearrange("p (blk wo ws) -> p blk wo ws", blk=60, wo=32, ws=4)
                P1 = work.tile([128, 1920], F32)
                nc.vector.tensor_tensor(P1[:], Xr[:, :, :, 0], Xr[:, :, :, 1], ADD)
                nc.vector.tensor_tensor(P1[:], P1[:], Xr[:, :, :, 2], ADD)
                nc.vector.tensor_tensor(P1[:], P1[:], Xr[:, :, :, 3], ADD)

                # ---- H-pool: sum hs -> P2 (do,di,wo) ----
                P1r = P1[:].rearrange("p (dodi hs wo) -> p dodi hs wo", dodi=15, hs=4, wo=32)
                P2 = work.tile([128, 480], F32)
                nc.vector.tensor_tensor(P2[:], P1r[:, :, 0], P1r[:, :, 1], ADD)
                nc.vector.tensor_tensor(P2[:], P2[:], P1r[:, :, 2], ADD)
                nc.vector.tensor_tensor(P2[:], P2[:], P1r[:, :, 3], ADD)

                # ---- D-pool: sum di -> Xd[p, (dp=do+1)*32+wo], dp=0,6 zero pads ----
                P2r = P2[:].rearrange("p (do di wo) -> p do di wo", do=5, di=3, wo=32)
                Xd = work.tile([128, 224], F32)
                nc.gpsimd.memset(Xd[:, 0:32], 0)
                nc.gpsimd.memset(Xd[:, 192:224], 0)
                nc.vector.tensor_tensor(Xd[:, 32:192], P2r[:, :, 0], P2r[:, :, 1], ADD)
                nc.vector.tensor_tensor(Xd[:, 32:192], Xd[:, 32:192], P2r[:, :, 2], ADD)

                # ---- depth conv (along do, free axis) ----
                m = work.tile([128, 160], F32)
                m2 = work.tile([128, 160], F32)
                nc.vector.tensor_tensor(m[:], wd0, Xd[:, 0:160], MULT)
                nc.vector.tensor_tensor(m2[:], wd1, Xd[:, 32:192], MULT)
                nc.vector.tensor_tensor(m[:], m[:], m2[:], ADD)
                nc.vector.tensor_tensor(m2[:], wd2, Xd[:, 64:224], MULT)
                nc.vector.tensor_tensor(m[:], m[:], m2[:], ADD)
                nc.vector.tensor_tensor(m[:], m[:], bd, ADD)
                # relu -> lon-padded tile Yd[p, do*34 + (wp=wo+1)]
                Yd = work.tile([128, 170], F32)
                Ydr = Yd[:].rearrange("p (do wp) -> p do wp", do=5, wp=34)
                nc.gpsimd.memset(Ydr[:, :, 0], 0)
                nc.gpsimd.memset(Ydr[:, :, 33], 0)
                nc.vector.tensor_scalar_max(Ydr[:, :, 1:33], m[:], 0.0)

                # ---- lon conv (along wo, free axis) ----
                nc.vector.tensor_tensor(m[:], vl0, Ydr[:, :, 0:32], MULT)
                nc.vector.tensor_tensor(m2[:], vl1, Ydr[:, :, 1:33], MULT)
                nc.vector.tensor_tensor(m[:], m[:], m2[:], ADD)
                nc.vector.tensor_tensor(m2[:], vl2, Ydr[:, :, 2:34], MULT)
                nc.vector.tensor_tensor(m[:], m[:], m2[:], ADD)
                nc.vector.tensor_tensor(m[:], m[:], blon, ADD)
                Yl = work.tile([128, 160], F32)
                nc.vector.tensor_scalar_max(Yl[:], m[:], 0.0)

                # ---- lat conv (along ho = partition axis; shifts via PE) ----
                Sup = psum.tile([128, 160], F32)
                Sdn = psum.tile([128, 160], F32)
                nc.tensor.matmul(Sup[:], shift_up, Yl[:], start=True, stop=True)
                nc.tensor.matmul(Sdn[:], shift_dn, Yl[:], start=True, stop=True)
                nc.vector.tensor_tensor(m[:], ul0, Sdn[:], MULT)
                nc.vector.tensor_tensor(m2[:], ul1, Yl[:], MULT)
                nc.vector.tensor_tensor(m[:], m[:], m2[:], ADD)
                nc.vector.tensor_tensor(m2[:], ul2, Sup[:], MULT)
                nc.vector.tensor_tensor(m[:], m[:], m2[:], ADD)
                nc.vector.tensor_tensor(m[:], m[:], blat, ADD)
                Y3 = work.tile([128, 160], F32)
                nc.vector.tensor_scalar_max(Y3[:], m[:], 0.0)

                # ---- upsample 3x4x4 into OUT[p, (i,a,c,(k,e))] ----
                OUT = outp.tile([128, 7680], F32)
                OUTr = OUT[:].rearrange(
                    "p (i a c k e) -> p i a c k e", i=5, a=3, c=4, k=32, e=4)
                Y3b = Y3[:].rearrange("p (i k) -> p i k", i=5).unsqueeze(2) \
                           .broadcast_to([128, 5, 4, 32])
                for e in range(4):
                    nc.scalar.copy(OUTr[:, :, 0, :, :, e], Y3b)
                OUTs = OUT[:].rearrange("p (i s) -> p i s", i=5, s=1536)
                nc.scalar.copy(OUTs[:, :, 512:1024], OUTs[:, :, 0:512])
                nc.scalar.copy(OUTs[:, :, 1024:1536], OUTs[:, :, 0:512])

                # ---- store ----
                for bl in range(B_GRP):
                    nc.sync.dma_start(
                        bass.AP(y, off + bl * BSTRIDE, list(_B_AP)),
                        OUT[bl * 16:(bl + 1) * 16, :])

    nc.compile()
    return nc


_NC_CACHE = {}


def _get_nc(reps: int = 1):
    if reps not in _NC_CACHE:
        _NC_CACHE[reps] = build_nc(reps)
    return _NC_CACHE[reps]


def kernel(x, w_depth, b_depth, w_lon, b_lon, w_lat, b_lat, reps: int = 1,
           **run_kwargs):
    cst = _pack_consts(w_depth, b_depth, w_lon, b_lon, w_lat, b_lat)
    xf = np.ascontiguousarray(np.asarray(x), dtype=np.float32).reshape(N_CORES, CORE_ELEMS)
    in_maps = [{"x": xf[c], "c": cst} for c in range(N_CORES)]
    nc = _get_nc(reps)
    res = run_bass_kernel_spmd(nc, in_maps, core_ids=list(range(N_CORES)), **run_kwargs)
    out = np.stack([r["y"] for r in res.results], axis=0)
    out = out.reshape(B, 15, 64, 128, 1)
    if run_kwargs:
        kernel.last_results = res
    return out

